# revision 1
# baseline (speedup 1.0000x reference)
"""Trainium2 Bass kernel for CustomGATConv (dense masked attention GNN layer).

  H = X @ W + b                       [8192, 64]
  S = H @ H.T ; S = where(A>0, S, -1e9)
  out = relu(softmax(S, -1) @ H)      [8192, 64]

Sharding: rows of the score matrix across 8 cores (1024 rows each).
Each core redundantly computes H (tiny) and processes its row block.

v2 design (from trace analysis of the fp32 baseline):
  - all inputs ride the two HWDGE rings (sync + scalar dma_start); the
    baseline's gpsimd (software-DGE) xt load serialized descriptor issue
    and idled the whole chip for ~100 us before the first matmul.
  - score + out matmuls in bf16 (1 cyc/row vs fp32's 4): H is computed
    in f32r (also 1 cyc/row, fp32 bit layout) and rounded to bf16 for
    the N x N part only.  The diagonal path (d_r = |h_r|^2, the two-term
    softmax merge) stays fp32, so the numerically-sensitive crossover
    rows keep reference accuracy; bf16 noise only perturbs off-diagonal
    attention weights, which average out over ~4096 neighbors.
  - masks arrive PRE-TRANSPOSED from the host as bf16 {0,1} tiles
    (the xbar-transpose DMAs and the dead hi16 "junk" stream are gone),
    and the mask is applied POST-exp as one 2x-rate bf16 DVE multiply
    instead of the 1x-rate fp32 scalar_tensor_tensor on PSUM.
  - exp runs on [128,1024] PSUM tiles ((N+352)/1.2 ns amortization).
  - H is built in 16 column chunks with per-chunk tiles so the Tile
    scheduler overlaps phase 1 with mask prefetch and loop start; the
    H.T duplicate (for tile_position score pairing) and the row-major
    [H_j|1] tiles are produced by SBUF->SBUF DMAs (dup) and xbar
    transpose DMAs (hsb) instead of PE transposes + ACT copies.
  - single [65, M] PSUM accumulator (K-extended with a ones column for
    the row sums); tail uses reciprocal_approx_fast and hoists the
    diag-score prep (emitted pre-loop) so only ~12 us remains serial.
"""

import sys
import numpy as np

for _p in ("/opt/trn_rl_repo",):
    if _p not in sys.path:
        sys.path.insert(0, _p)

import concourse.bass as bass
import concourse.tile as tile
from concourse import bacc, mybir
from concourse.bass_utils import run_bass_kernel_spmd

N = 8192          # nodes
D = 200           # in dim
F = 64            # out dim
NCORES = 8
M = N // NCORES   # 1024 rows per core
P = 128           # partitions
C_SHIFT = 64.0    # global softmax shift for off-diagonal scores

f32 = mybir.dt.float32
f32r = mybir.dt.float32r
bf16 = mybir.dt.bfloat16
f16 = mybir.dt.float16
i32 = mybir.dt.int32
AF = mybir.ActivationFunctionType
ALU = mybir.AluOpType

NW = N + F        # xt cols: [X.T | W] fused so the whole load is one stream


def build_kernel(nc, outT, xt, am, adiag):
    from contextlib import ExitStack

    with ExitStack() as ctx:
        tc = nc._tc
        const = ctx.enter_context(tc.tile_pool(name="const", bufs=1))
        aux = ctx.enter_context(tc.tile_pool(name="aux", bufs=2, space="PSUM"))
        ps_pool = ctx.enter_context(tc.tile_pool(name="ps", bufs=2, space="PSUM"))
        ps_out = ctx.enter_context(tc.tile_pool(name="ps_out", bufs=1, space="PSUM"))
        fix = ctx.enter_context(tc.tile_pool(name="fix", bufs=1))

        # persistent tiles.  scores run in fp16 (1 cyc/row like bf16, but
        # 8x less rounding noise); e must be bf16 for range (e^35 overflows
        # fp16), so the out-matmul lhsT [H_j | 1] is converted fp16->bf16.
        hbc = [const.tile([2 * F, 512], f16, tag=f"hb{k}", name=f"hb{k}")
               for k in range(16)]          # H.T fp16, rows 64..127 duplicate
        hst = [const.tile([P, 4 * F], f16, tag=f"ht{k}", name=f"ht{k}")
               for k in range(16)]          # xbar-transpose staging (aligned)
        hsbc = [const.tile([P, 4 * (F + 1)], bf16, tag=f"hs{k}", name=f"hs{k}")
                for k in range(16)]         # per j-tile row-major [H_j | 1]
        hto = const.tile([F, M], f32)       # own-rows H, fp32 (diag path)
        cbias = const.tile([P, 1], f32)     # -C bias for the exp
        nc.vector.memset(cbias[:], -C_SHIFT)
        for k in range(16):
            h3 = hsbc[k][:].rearrange("p (a b) -> p a b", b=F + 1)
            nc.vector.memset(h3[:, :, F : F + 1], 1.0)

        # ---------------- phase 1: load [X.T|W]; compute H chunks ----------
        # xt = [X.T ; ones | W ; b] (201 x 8256) so K = 128 + 73 covers
        # X@W + b with the weight block as an SBUF slice of the same tiles.
        with tc.tile_pool(name="xtp", bufs=1) as xtp:
            xt1 = xtp.tile([P, NW], f32r)
            nc.scalar.dma_start(xt1[:], xt[0:P, :])
            xt2 = xtp.tile([D + 1 - P, NW], f32r)
            nc.scalar.dma_start(xt2[:], xt[P : D + 1, :])

            for c in range(16):
                ps = aux.tile([F, 512], f32, tag="aux", name=f"ph1_{c}")
                s = bass.ts(c, 512)
                nc.tensor.matmul(ps[:], xt1[:, N : N + F], xt1[:, s],
                                 start=True, stop=False)
                nc.tensor.matmul(ps[:], xt2[:, N : N + F], xt2[:, s],
                                 start=False, stop=True)
                nc.vector.tensor_copy(hbc[c][0:F, :], ps[:])
                nc.scalar.dma_start(hbc[c][F : 2 * F, :], hbc[c][0:F, :])
                if c < 2:
                    nc.vector.tensor_copy(hto[:, s], ps[:])
                h4 = hst[c][:].rearrange("p (a b) -> p a b", b=F)
                for q in range(4):
                    # xbar transpose needs 4B-aligned outputs, so it lands
                    # in the 64-stride staging tile, not the 65-stride hsbc
                    nc.scalar.dma_start_transpose(
                        h4[:, q, :], hbc[c][0:F, bass.ts(q, P)])
                h3 = hsbc[c][:].rearrange("p (a b) -> p a b", b=F + 1)
                nc.gpsimd.tensor_copy(h3[:, :, 0:F], h4[:])

        # ---------------- tail-pre: diag-score prep (overlaps the loop) ----
        # d_r = |h_r|^2 (diag score), a_r = A[r,r]
        adi = fix.tile([1, M], i32)
        nc.scalar.dma_start(adi[:], adiag[:])
        ad = fix.tile([1, M], f32)
        nc.vector.tensor_copy(ad[:], adi[:])
        htsq = fix.tile([F, M], f32)
        nc.vector.tensor_mul(htsq[:], hto[:], hto[:])
        ones64 = fix.tile([F, 1], f32)
        nc.vector.memset(ones64[:], 1.0)
        dsq = fix.tile([1, M], f32)
        for hi in range(2):
            psd = aux.tile([1, 512], f32, tag="aux", name=f"dsq{hi}")
            nc.tensor.matmul(psd[:], ones64[:], htsq[:, bass.ts(hi, 512)],
                             start=True, stop=True)
            nc.vector.tensor_copy(dsq[:, bass.ts(hi, 512)], psd[:])
        # t1 = a*(d - C + 100) - 100  (== d-C where diag present, else -100)
        t1 = fix.tile([1, M], f32)
        nc.vector.scalar_tensor_tensor(t1[:], dsq[:], 100.0 - C_SHIFT, ad[:],
                                       ALU.add, ALU.mult)
        nc.vector.tensor_scalar_add(t1[:], t1[:], -100.0)
        mmx = fix.tile([1, M], f32)
        nc.vector.tensor_scalar_max(mmx[:], t1[:], 0.0)
        scm = fix.tile([1, M], f32)   # e^{-m}: scale for the off-diag partials
        nc.scalar.activation(scm[:], mmx[:], AF.Exp, scale=-1.0)
        scd = fix.tile([1, M], f32)   # e^{t1-m}: scale for the diag term
        nc.vector.tensor_sub(scd[:], t1[:], mmx[:])
        nc.scalar.activation(scd[:], scd[:], AF.Exp)

        # ---------------- phase 2: main attention loop ---------------------
        atp = ctx.enter_context(tc.tile_pool(name="at", bufs=8))
        ep = ctx.enter_context(tc.tile_pool(name="ep", bufs=4))

        po = ps_out.tile([F + 1, M], f32)
        HALVES = (slice(0, 512), slice(512, M))

        for t in range(32):
            j0, j1 = 2 * t, 2 * t + 1
            at0 = atp.tile([P, M], bf16, tag="at", name="at0")
            nc.sync.dma_start(at0[:], am[j0])
            at1 = atp.tile([P, M], bf16, tag="at", name="at1")
            nc.sync.dma_start(at1[:], am[j1])

            for j, at in ((j0, at0), (j1, at1)):
                if j < 8:  # rotated frame: diagonal crosses tiles 0..7
                    nc.gpsimd.affine_select(
                        at[:], at[:], pattern=[[-1, M]], base=j * P,
                        channel_multiplier=1, compare_op=ALU.not_equal,
                        fill=0.0)

            l0 = hbc[j0 // 4][0:F, bass.ts(j0 % 4, P)]
            l1 = hbc[j1 // 4][F : 2 * F, bass.ts(j1 % 4, P)]
            ps0 = ps_pool.tile([P, M], f32, tag="ps", name="ps0")
            ps1 = ps_pool.tile([P, M], f32, tag="ps", name="ps1")
            # all four score matmuls adjacent; (0,0)/(64,0) pairs run
            # concurrently on distinct PE row-groups
            for hi, half in enumerate(HALVES):
                nc.tensor.matmul(ps0[:, half], l0, hbc[hi][0:F, :],
                                 start=True, stop=True, tile_position=(0, 0))
                nc.tensor.matmul(ps1[:, half], l1, hbc[hi][F : 2 * F, :],
                                 start=True, stop=True, tile_position=(64, 0))

            st, sp = (t == 0), (t == 31)
            for j, at, ps in ((j0, at0, ps0), (j1, at1, ps1)):
                if j < 8:
                    # diag scores (|h_r|^2, up to ~190) would overflow bf16
                    # after exp (bf16 max = e^88.7); off-diag max is 99.6 so
                    # the clamp only touches the diagonal, which the mask
                    # zeroes and the tail re-adds exactly
                    nc.vector.tensor_scalar_min(ps[:], ps[:], 150.0)
                e = ep.tile([P, M], bf16, tag="e", name="e")
                nc.scalar.activation(e[:], ps[:], AF.Exp, bias=cbias[:])
                nc.vector.tensor_mul(e[:], e[:], at[:])
                lh = hsbc[j // 4][:, (j % 4) * (F + 1) : (j % 4 + 1) * (F + 1)]
                nc.tensor.matmul(po[:, HALVES[0]], lh, e[:, HALVES[0]],
                                 start=st and j == j0, stop=sp and j == j1,
                                 skip_group_check=True)
                nc.tensor.matmul(po[:, HALVES[1]], lh, e[:, HALVES[1]],
                                 start=st and j == j0, stop=sp and j == j1,
                                 skip_group_check=True)

        # ---------------- phase 3: merge + normalize -----------------------
        posb = fix.tile([F, M], f32)
        nc.scalar.copy(posb[:], po[0:F, :])
        esum = fix.tile([1, M], f32)
        nc.scalar.copy(esum[:], po[F : F + 1, :])
        den = fix.tile([1, M], f32)
        nc.vector.tensor_mul(den[:], esum[:], scm[:])
        nc.vector.tensor_add(den[:], den[:], scd[:])
        rden = fix.tile([1, M], f32)
        nc.vector.reciprocal_approx_fast(rden[:], den[:])
        alpha = fix.tile([1, M], f32)
        nc.vector.tensor_mul(alpha[:], scm[:], rden[:])
        beta = fix.tile([1, M], f32)
        nc.vector.tensor_mul(beta[:], scd[:], rden[:])

        # broadcast alpha/beta across 64 partitions via K=1 matmul with ones
        ones_row = fix.tile([1, F], f32)
        nc.vector.memset(ones_row[:], 1.0)
        res = fix.tile([F, M], f32)
        res2 = fix.tile([F, M], f32)
        for hi, half in enumerate(HALVES):
            ab = aux.tile([F, 512], f32, tag="aux", name=f"ab{hi}")
            nc.tensor.matmul(ab[:], ones_row[:], alpha[:, half],
                             start=True, stop=True)
            nc.vector.tensor_mul(res[:, half], posb[:, half], ab[:])
            bb = aux.tile([F, 512], f32, tag="aux", name=f"bb{hi}")
            nc.tensor.matmul(bb[:], ones_row[:], beta[:, half],
                             start=True, stop=True)
            nc.vector.tensor_mul(res2[:, half], hto[:, half], bb[:])
        nc.vector.tensor_add(res[:], res[:], res2[:])
        osb = fix.tile([F, M], f32)
        nc.scalar.activation(osb[:], res[:], AF.Relu)
        nc.sync.dma_start(outT[:], osb[:])


_NC_CACHE = {}


def get_compiled():
    if "nc" not in _NC_CACHE:
        nc = bacc.Bacc("TRN2", target_bir_lowering=False, debug=False,
                       enable_asserts=True, num_devices=NCORES)
        xt = nc.dram_tensor("xt", [D + 1, NW], f32r, kind="ExternalInput").ap()
        am = nc.dram_tensor("am", [64, P, M], bf16, kind="ExternalInput").ap()
        adiag = nc.dram_tensor("adiag", [1, M], i32, kind="ExternalInput").ap()
        outT = nc.dram_tensor("outT", [F, M], f32, kind="ExternalOutput").ap()
        with tile.TileContext(nc) as tc:
            nc._tc = tc
            build_kernel(nc, outT, xt, am, adiag)
        nc.compile()
        _NC_CACHE["nc"] = nc
    return _NC_CACHE["nc"]


def make_in_maps(X, A, W, b):
    import ml_dtypes
    X = np.ascontiguousarray(np.asarray(X, dtype=np.float32))
    A = np.asarray(A)
    if A.dtype != np.int32:
        A = A.astype(np.int32)
    W = np.asarray(W, dtype=np.float32)
    b = np.asarray(b, dtype=np.float32).reshape(1, F)
    wb = np.concatenate([W, b], axis=0)  # [201, 64]
    XT = np.concatenate([X.T, np.ones((1, N), np.float32)], axis=0)  # [201, N]
    rng = np.arange(M)
    in_maps = []
    for c in range(NCORES):
        r0 = c * M
        xt_c = np.ascontiguousarray(
            np.concatenate([np.roll(XT, -r0, axis=1), wb], axis=1))
        blk = np.roll(A[r0 : r0 + M], -r0, axis=1)          # [M, N]
        am = ((blk.T.reshape(64, P, M) > 0).astype(np.uint16) * 0x3F80)
        am = np.ascontiguousarray(am).view(ml_dtypes.bfloat16)
        adiag = A[r0 + rng, r0 + rng].reshape(1, M).astype(np.int32)
        in_maps.append({"xt": xt_c, "am": am, "adiag": adiag})
    return in_maps


def kernel(X, A, W, b):
    nc = get_compiled()
    in_maps = make_in_maps(X, A, W, b)
    res = run_bass_kernel_spmd(nc, in_maps, list(range(NCORES)))
    outTs = [res.results[c]["outT"] for c in range(NCORES)]
    return np.ascontiguousarray(np.concatenate(outTs, axis=1).T)



# revision 3
# speedup vs baseline: 1.9765x; 1.9765x over previous
"""Trainium2 Bass kernel for CustomGATConv (dense masked attention GNN layer).

  H = X @ W + b                       [8192, 64]
  S = H @ H.T ; S = where(A>0, S, -1e9)
  out = relu(softmax(S, -1) @ H)      [8192, 64]

Sharding: rows of the score matrix across 8 cores (1024 rows each).
Each core redundantly computes H (tiny) and processes its row block.

v3 design (from trace analysis of v2's 407us):
  v2's masks (17MB/core, the dominant stream) didn't START loading until
  t=215us: their sync-ring DMAs were queued behind phase 1's congested
  scalar ring (6.6MB fp32 xt + 64 serial 1.2us DMA transposes) through
  the shared 8-deep HWDGE completion-semaphore round-robin. Fixes:
  - mask pair-tiles [128, 2048] bf16 (4KB/partition descriptors), issued
    FIRST in program order on the otherwise-idle sync ring; 6-deep
    prefetch sustained through the loop. Diagonal zeroed on host (kills
    the 8 gpsimd affine_selects).
  - xt in fp16 (3.3MB) as 8 column-chunk DMAs on the scalar ring, so H
    chunks start as soon as their chunk lands; [X.T|W] split into
    separate xt/wb tensors so the weights arrive first.
  - the 64 scalar-ring DMA transposes (77us) for row-major [H_j|1] are
    replaced by PE transposes (fp16, via identity) into fp16 PSUM.
  - scores matmul straight into fp16 PSUM pair tiles [128, 2048] (1 PSUM
    bank each, single-shot K=64 so no f32 accumulation needed): one exp
    per PAIR on ACT ((2048+344)/1.2us) and 3-deep PSUM pipelining beside
    the 2-bank out accumulator.
  - post-exp mask multiply as one pair-wide [128,2048] bf16 DVE op (2x).
  - ACT (exp) is the steady-state pacer at ~2.0us/pair; PE ~1.9us/pair
    (4x512-row score streams + 4x512 out streams + weight loads).
"""

import sys
import numpy as np

for _p in ("/opt/trn_rl_repo",):
    if _p not in sys.path:
        sys.path.insert(0, _p)

import concourse.bass as bass
import concourse.tile as tile
from concourse import bacc, mybir
from concourse.bass_utils import run_bass_kernel_spmd
from concourse.masks import make_identity

N = 8192          # nodes
D = 200           # in dim
F = 64            # out dim
NCORES = 8
M = N // NCORES   # 1024 rows per core
P = 128           # partitions
C_SHIFT = 64.0    # global softmax shift for off-diagonal scores
PREFETCH = 6      # mask pair tiles in flight

f32 = mybir.dt.float32
bf16 = mybir.dt.bfloat16
f16 = mybir.dt.float16
i32 = mybir.dt.int32
AF = mybir.ActivationFunctionType
ALU = mybir.AluOpType


def build_kernel(nc, outT, xt, wb, am, adiag):
    from contextlib import ExitStack

    with ExitStack() as ctx:
        tc = nc._tc
        const = ctx.enter_context(tc.tile_pool(name="const", bufs=1))
        fix = ctx.enter_context(tc.tile_pool(name="fix", bufs=1))

        # persistent tiles.  scores run in fp16 (1 cyc/row); e must be bf16
        # for range (e^35 overflows fp16).
        hbc = [const.tile([P, 512], f16, tag=f"hb{k}", name=f"hb{k}")
               for k in range(16)]          # H.T fp16, rows 64..127 duplicate
        hsbc = [const.tile([P, 4 * (F + 1)], bf16, tag=f"hs{k}", name=f"hs{k}")
                for k in range(16)]         # per j-tile row-major [H_j | 1]
        hto = const.tile([F, M], f32)       # own-rows H, fp32 (diag path)
        cbias = const.tile([P, 1], f32)     # -C bias for the exp
        ident = const.tile([F, F], f16)     # PE-transpose identity

        # ---------------- mask prefetch: first in program order -----------
        atp = ctx.enter_context(tc.tile_pool(name="at", bufs=PREFETCH))
        at_tiles = {}

        def issue_at(t):
            a = atp.tile([P, 2 * M], bf16, tag="at", name=f"at{t % PREFETCH}")
            nc.sync.dma_start(a[:], am[t])
            at_tiles[t] = a

        for t in range(PREFETCH):
            issue_at(t)

        nc.vector.memset(cbias[:], -C_SHIFT)
        make_identity(nc, ident[:])
        for k in range(16):
            h3 = hsbc[k][:].rearrange("p (a b) -> p a b", b=F + 1)
            nc.vector.memset(h3[:, :, F : F + 1], 1.0)

        # ---------------- phase 1: load X.T/W chunks; compute H ------------
        # H.T chunks via (W|b).T @ (X|1) (K=201 split 128+73); row-major
        # [H_j|1] via PE transposes of the H.T chunks.
        adi = fix.tile([1, M], i32)
        htsq = fix.tile([F, M], f32)
        ones64 = fix.tile([F, 1], f32)
        dsq = fix.tile([1, M], f32)
        with tc.tile_pool(name="xtp", bufs=1) as xtp, \
             tc.tile_pool(name="psA", bufs=2, space="PSUM") as psA, \
             tc.tile_pool(name="psT", bufs=2, space="PSUM") as psT:
            wt1 = xtp.tile([P, F], f16)
            nc.scalar.dma_start(wt1[:], wb[0:P, :])
            wt2 = xtp.tile([D + 1 - P, F], f16)
            nc.scalar.dma_start(wt2[:], wb[P : D + 1, :])
            nc.scalar.dma_start(adi[:], adiag[:])
            xt1c = []
            xt2c = []
            for c4 in range(4):
                x1 = xtp.tile([P, 2048], f16, tag="xt1", name=f"xt1_{c4}")
                nc.scalar.dma_start(x1[:], xt[0:P, bass.ts(c4, 2048)])
                xt1c.append(x1)
                x2 = xtp.tile([D + 1 - P, 2048], f16, tag="xt2",
                              name=f"xt2_{c4}")
                nc.scalar.dma_start(x2[:], xt[P : D + 1, bass.ts(c4, 2048)])
                xt2c.append(x2)

            for c in range(16):
                ps = psA.tile([F, 512], f32, tag="psA", name=f"ph1_{c}")
                s = bass.ts(c % 4, 512)
                nc.tensor.matmul(ps[:], wt1[:], xt1c[c // 4][:, s],
                                 start=True, stop=False)
                nc.tensor.matmul(ps[:], wt2[:], xt2c[c // 4][:, s],
                                 start=False, stop=True)
                nc.vector.tensor_copy(hbc[c][0:F, :], ps[:])
                if c < 2:
                    nc.scalar.copy(hto[:, bass.ts(c, 512)], ps[:])
                # dup rows 64..127 for the (64,0)-quadrant operands
                nc.scalar.dma_start(hbc[c][F : 2 * F, :], hbc[c][0:F, :])
                pst = psT.tile([P, 4 * F], f16, tag="psT", name=f"tr_{c}")
                for q in range(4):
                    nc.tensor.transpose(pst[:, bass.ts(q, F)],
                                        hbc[c][0:F, bass.ts(q, P)], ident[:])
                h3 = hsbc[c][:].rearrange("p (a b) -> p a b", b=F + 1)
                p3 = pst[:].rearrange("p (a b) -> p a b", b=F)
                nc.scalar.copy(h3[:, :, 0:F], p3[:])

            # d_r = |h_r|^2 partials while psA is still open
            nc.vector.tensor_mul(htsq[:], hto[:], hto[:])
            nc.vector.memset(ones64[:], 1.0)
            for hi in range(2):
                psd = psA.tile([1, 512], f32, tag="psA", name=f"dsq{hi}")
                nc.tensor.matmul(psd[:], ones64[:], htsq[:, bass.ts(hi, 512)],
                                 start=True, stop=True)
                nc.vector.tensor_copy(dsq[:, bass.ts(hi, 512)], psd[:])

        # ---------------- tail-pre: diag-score merge scales ----------------
        # t1 = a*(d - C + 100) - 100  (== d-C where diag present, else -100)
        ad = fix.tile([1, M], f32)
        nc.vector.tensor_copy(ad[:], adi[:])
        t1 = fix.tile([1, M], f32)
        nc.vector.scalar_tensor_tensor(t1[:], dsq[:], 100.0 - C_SHIFT, ad[:],
                                       ALU.add, ALU.mult)
        nc.vector.tensor_scalar_add(t1[:], t1[:], -100.0)
        mmx = fix.tile([1, M], f32)
        nc.vector.tensor_scalar_max(mmx[:], t1[:], 0.0)
        scm = fix.tile([1, M], f32)   # e^{-m}: scale for the off-diag partials
        nc.scalar.activation(scm[:], mmx[:], AF.Exp, scale=-1.0)
        scd = fix.tile([1, M], f32)   # e^{t1-m}: scale for the diag term
        nc.vector.tensor_sub(scd[:], t1[:], mmx[:])
        nc.scalar.activation(scd[:], scd[:], AF.Exp)

        # ---------------- phase 2: main attention loop ---------------------
        ep = ctx.enter_context(tc.tile_pool(name="ep", bufs=3))
        psP = ctx.enter_context(tc.tile_pool(name="ps", bufs=3, space="PSUM"))
        ps_out = ctx.enter_context(tc.tile_pool(name="po", bufs=1,
                                                space="PSUM"))

        po = ps_out.tile([F + 1, M], f32, tag="po", name="po")
        HALVES = (slice(0, 512), slice(512, M))

        for t in range(32):
            if t + PREFETCH < 32:
                issue_at(t + PREFETCH)
            at = at_tiles.pop(t)
            j0, j1 = 2 * t, 2 * t + 1
            l0 = hbc[j0 // 4][0:F, bass.ts(j0 % 4, P)]
            l1 = hbc[j1 // 4][F : 2 * F, bass.ts(j1 % 4, P)]
            ps0 = psP.tile([P, M], f32, tag="ps", name="ps0")
            ps1 = psP.tile([P, M], f32, tag="ps", name="ps1")
            for hi in range(2):
                nc.tensor.matmul(ps0[:, bass.ts(hi, 512)], l0,
                                 hbc[hi][0:F, :],
                                 start=True, stop=True, tile_position=(0, 0))
                nc.tensor.matmul(ps1[:, bass.ts(hi, 512)], l1,
                                 hbc[hi][F : 2 * F, :],
                                 start=True, stop=True, tile_position=(64, 0))
            e = ep.tile([P, 2 * M], bf16, tag="e", name="e")
            for hi, psx in ((0, ps0), (1, ps1)):
                if t < 4:
                    # diag scores (|h_r|^2, up to ~190) would overflow bf16
                    # after exp (bf16 max = e^88.7); off-diag max is ~99.6 so
                    # the clamp only touches the diagonal, which the mask
                    # zeroes and the tail re-adds exactly
                    nc.vector.tensor_scalar_min(psx[:], psx[:], 150.0)
                nc.scalar.activation(e[:, bass.ts(hi, M)], psx[:], AF.Exp,
                                     bias=cbias[:])
            nc.vector.tensor_mul(e[:], e[:], at[:])
            st, sp = (t == 0), (t == 31)
            for j, c0 in ((j0, 0), (j1, M)):
                lh = hsbc[j // 4][:, (j % 4) * (F + 1) : (j % 4 + 1) * (F + 1)]
                for hi, half in enumerate(HALVES):
                    nc.tensor.matmul(po[:, half], lh,
                                     e[:, c0 + hi * 512 : c0 + (hi + 1) * 512],
                                     start=st and j == j0, stop=sp and j == j1,
                                     skip_group_check=True)

        # ---------------- phase 3: merge + normalize -----------------------
        posb = fix.tile([F, M], f32)
        nc.scalar.copy(posb[:], po[0:F, :])
        esum = fix.tile([1, M], f32)
        nc.scalar.copy(esum[:], po[F : F + 1, :])
        den = fix.tile([1, M], f32)
        nc.vector.tensor_mul(den[:], esum[:], scm[:])
        nc.vector.tensor_add(den[:], den[:], scd[:])
        rden = fix.tile([1, M], f32)
        nc.vector.reciprocal_approx_fast(rden[:], den[:])
        alpha = fix.tile([1, M], f32)
        nc.vector.tensor_mul(alpha[:], scm[:], rden[:])
        beta = fix.tile([1, M], f32)
        nc.vector.tensor_mul(beta[:], scd[:], rden[:])

        # broadcast alpha/beta across 64 partitions via K=1 matmul with ones
        ones_row = fix.tile([1, F], f32)
        nc.vector.memset(ones_row[:], 1.0)
        res = fix.tile([F, M], f32)
        res2 = fix.tile([F, M], f32)
        for hi, half in enumerate(HALVES):
            ab = ps_out.tile([F, 512], f32, tag="po", name=f"ab{hi}")
            nc.tensor.matmul(ab[:], ones_row[:], alpha[:, half],
                             start=True, stop=True)
            nc.vector.tensor_mul(res[:, half], posb[:, half], ab[:])
            bb = ps_out.tile([F, 512], f32, tag="po", name=f"bb{hi}")
            nc.tensor.matmul(bb[:], ones_row[:], beta[:, half],
                             start=True, stop=True)
            nc.vector.tensor_mul(res2[:, half], hto[:, half], bb[:])
        nc.vector.tensor_add(res[:], res[:], res2[:])
        osb = fix.tile([F, M], f32)
        nc.scalar.activation(osb[:], res[:], AF.Relu)
        nc.sync.dma_start(outT[:], osb[:])


_NC_CACHE = {}


def get_compiled():
    if "nc" not in _NC_CACHE:
        nc = bacc.Bacc("TRN2", target_bir_lowering=False, debug=False,
                       enable_asserts=True, num_devices=NCORES)
        xt = nc.dram_tensor("xt", [D + 1, N], f16, kind="ExternalInput").ap()
        wb = nc.dram_tensor("wb", [D + 1, F], f16, kind="ExternalInput").ap()
        am = nc.dram_tensor("am", [32, P, 2 * M], bf16,
                            kind="ExternalInput").ap()
        adiag = nc.dram_tensor("adiag", [1, M], i32, kind="ExternalInput").ap()
        outT = nc.dram_tensor("outT", [F, M], f32, kind="ExternalOutput").ap()
        with tile.TileContext(nc) as tc:
            nc._tc = tc
            build_kernel(nc, outT, xt, wb, am, adiag)
        nc.compile()
        _NC_CACHE["nc"] = nc
    return _NC_CACHE["nc"]


def make_in_maps(X, A, W, b):
    import ml_dtypes
    X = np.asarray(X, dtype=np.float32)
    A = np.asarray(A)
    if A.dtype != np.int32:
        A = A.astype(np.int32)
    W = np.asarray(W, dtype=np.float32)
    b = np.asarray(b, dtype=np.float32).reshape(1, F)
    wb = np.ascontiguousarray(
        np.concatenate([W, b], axis=0).astype(np.float16))      # [201, 64]
    XT = np.concatenate([X.T, np.ones((1, N), np.float32)],
                        axis=0).astype(np.float16)              # [201, N]
    rng = np.arange(M)
    in_maps = []
    for c in range(NCORES):
        r0 = c * M
        xt_c = np.ascontiguousarray(np.roll(XT, -r0, axis=1))
        blk = np.roll(A[r0 : r0 + M], -r0, axis=1).copy()       # [M, N]
        adiag = blk[rng, rng].reshape(1, M).astype(np.int32)
        blk[rng, rng] = 0            # diag handled exactly by the fp32 tail
        # pair layout [32, 128, 2048]: am[t, p, i*M + m] = (A.T)[2t+i, p, m]
        am = ((blk.T.reshape(32, 2, P, M) > 0).astype(np.uint16) * 0x3F80)
        am = np.ascontiguousarray(
            am.transpose(0, 2, 1, 3).reshape(32, P, 2 * M))
        am = am.view(ml_dtypes.bfloat16)
        in_maps.append({"xt": xt_c, "wb": wb, "am": am, "adiag": adiag})
    return in_maps


def kernel(X, A, W, b):
    nc = get_compiled()
    in_maps = make_in_maps(X, A, W, b)
    res = run_bass_kernel_spmd(nc, in_maps, list(range(NCORES)))
    outTs = [res.results[c]["outT"] for c in range(NCORES)]
    return np.ascontiguousarray(np.concatenate(outTs, axis=1).T)


# revision 17
# speedup vs baseline: 2.0408x; 1.0325x over previous
"""Trainium2 Bass kernel for CustomGATConv (dense masked attention GNN layer).

  H = X @ W + b                       [8192, 64]
  S = H @ H.T ; S = where(A>0, S, -1e9)
  out = relu(softmax(S, -1) @ H)      [8192, 64]

Sharding: rows of the score matrix across 8 cores (1024 rows each).
Each core redundantly computes H (tiny) and processes its row block.

v5 design (from v3/v4 traces):
  v3 was mask-DMA-bound: a HWDGE queue dispatches only ~23 descriptors/us
  (all 16 DMA engines sat ~73% idle behind the queue head), so bandwidth
  scales with descriptor SIZE.  Mask bit-packing is off the table: the
  STT ISA only chains basic arith ops (no bitwise/mod/is_ge before a
  mult), and one scalar op can't extract two independent bits.
  - masks stay bf16 {0,1} but in DRAM layout [8, 128, 8192]: each DMA
    moves 2MB with 16KB per-partition contiguous lines (128
    descriptors) -> ~4x the per-queue bandwidth of v3's 4KB lines.  All
    eight issued up front on the sync ring (the 3-deep tile ring
    naturally paces them); xt/wb ride the scalar ring as two whole-row
    DMAs.  Zero DMA issues during the loop.
  - H chunks (incl. the partition-64..127 duplicate via a second matmul
    at tile_position col 64 -- no SBUF-to-SBUF dup DMAs) interleave with
    the attention loop; PSUM-read copies split ACT (early chunks) / DVE.
  - out matmuls software-pipelined one pair behind the score matmuls so
    the PE never waits on exp/mask of the current pair.
  - ACT does exp only: 64 x [128,1024] ~ 73us, the steady-state pacer.
"""

import sys
import numpy as np

for _p in ("/opt/trn_rl_repo",):
    if _p not in sys.path:
        sys.path.insert(0, _p)

import concourse.bass as bass
import concourse.tile as tile
from concourse import bacc, mybir
from concourse.bass_utils import run_bass_kernel_spmd
from concourse.masks import make_identity

N = 8192          # nodes
D = 200           # in dim
F = 64            # out dim
NCORES = 8
M = N // NCORES   # 1024 rows per core
P = 128           # partitions
C_SHIFT = 64.0    # global softmax shift for off-diagonal scores

f32 = mybir.dt.float32
bf16 = mybir.dt.bfloat16
f16 = mybir.dt.float16
i16 = mybir.dt.int16
i32 = mybir.dt.int32
AF = mybir.ActivationFunctionType
ALU = mybir.AluOpType


def build_kernel(nc, outT, xt, wb, am, adiag):
    from contextlib import ExitStack

    with ExitStack() as ctx:
        tc = nc._tc
        const = ctx.enter_context(tc.tile_pool(name="const", bufs=1))
        fix = ctx.enter_context(tc.tile_pool(name="fix", bufs=1))
        atp = ctx.enter_context(tc.tile_pool(name="at", bufs=3))
        ep = ctx.enter_context(tc.tile_pool(name="ep", bufs=4))
        psP = ctx.enter_context(tc.tile_pool(name="ps", bufs=3, space="PSUM"))

        hbc = [const.tile([P, 512], f16, tag=f"hb{k}", name=f"hb{k}")
               for k in range(16)]          # H.T fp16, rows 64..127 duplicate
        hsbc = [const.tile([P, 4 * (F + 1)], bf16, tag=f"hs{k}", name=f"hs{k}")
                for k in range(16)]         # per j-tile row-major [H_j | 1]
        hto = const.tile([F, M], f32)       # own-rows H, fp32 (diag path)
        cbias = const.tile([P, 1], f32)     # -C bias for the exp
        ident = const.tile([F, F], f16)     # PE-transpose identity
        xt1 = const.tile([P, N], f16)
        xt2 = const.tile([D + 1 - P, N], f16)
        wt1 = const.tile([P, F], f16)
        wt2 = const.tile([D + 1 - P, F], f16)

        # ---- ring issue order.  sync: xt rows 0..127, then all four mask
        # ---- groups (2MB each, 16KB descriptor lines).  scalar: weights,
        # ---- adiag, xt rows 128..200.  Nothing issues during the loop.
        nc.sync.dma_start(xt1[:], xt[0:P, :])
        at_g = []
        for g in range(8):
            a = atp.tile([P, 8 * M], bf16, tag="at", name=f"atg{g % 3}")
            nc.sync.dma_start(a[:], am[g])
            at_g.append(a)
        nc.scalar.dma_start(wt1[:], wb[0:P, :])
        nc.scalar.dma_start(wt2[:], wb[P : D + 1, :])
        adi = fix.tile([1, M], i32)
        nc.scalar.dma_start(adi[:], adiag[:])
        nc.scalar.dma_start(xt2[:], xt[P : D + 1, :])

        nc.vector.memset(cbias[:], -C_SHIFT)
        make_identity(nc, ident[:])
        for k in range(16):
            h3 = hsbc[k][:].rearrange("p (a b) -> p a b", b=F + 1)
            nc.vector.memset(h3[:, :, F : F + 1], 1.0)

        # ---- phase 1 chunk emitters (chunk c = H.T cols 512c..512c+511) ----
        def emit_chunk_mm(c):
            ps = psP.tile([P, 512], f32, tag="ps", name=f"h{c}")
            s = bass.ts(c, 512)
            # rows 0..63 and the 64..127 duplicate, straight from the PE
            nc.tensor.matmul(ps[0:F, :], wt1[:], xt1[:, s],
                             start=True, stop=False)
            nc.tensor.matmul(ps[0:F, :], wt2[:], xt2[:, s],
                             start=False, stop=True)
            nc.tensor.matmul(ps[F : 2 * F, :], wt1[:], xt1[:, s],
                             start=True, stop=False)
            nc.tensor.matmul(ps[F : 2 * F, :], wt2[:], xt2[:, s],
                             start=False, stop=True)
            # early chunks ride the pre-loop-idle ACT engine, later ones DVE
            if c < 8:
                nc.scalar.copy(hbc[c][:], ps[:])
            else:
                nc.vector.tensor_copy(hbc[c][:], ps[:])
            if c < 2:
                nc.scalar.copy(hto[:, bass.ts(c, 512)], ps[0:F, :])

        def emit_chunk_tr(c):
            trp = psP.tile([P, 4 * F], f16, tag="ps", name=f"t{c}")
            for q in range(4):
                nc.tensor.transpose(trp[:, bass.ts(q, F)],
                                    hbc[c][0:F, bass.ts(q, P)], ident[:])
            h3 = hsbc[c][:].rearrange("p (a b) -> p a b", b=F + 1)
            p3 = trp[:].rearrange("p (a b) -> p a b", b=F)
            if c < 8:
                nc.scalar.copy(h3[:, :, 0:F], p3[:])
            else:
                nc.vector.tensor_copy(h3[:, :, 0:F], p3[:])

        for c in range(4):
            emit_chunk_mm(c)
        for c in range(4):
            emit_chunk_tr(c)

        # ---- diag-score prep: d_r = |h_r|^2, merge scales ----
        htsq = fix.tile([F, M], f32)
        nc.vector.tensor_mul(htsq[:], hto[:], hto[:])
        ones64 = fix.tile([F, 1], f32)
        nc.vector.memset(ones64[:], 1.0)
        dsq = fix.tile([1, M], f32)
        for hi in range(2):
            psd = psP.tile([P, 512], f32, tag="ps", name=f"dsq{hi}")
            nc.tensor.matmul(psd[0:1, :], ones64[:], htsq[:, bass.ts(hi, 512)],
                             start=True, stop=True)
            nc.vector.tensor_copy(dsq[:, bass.ts(hi, 512)], psd[0:1, :])
        ad = fix.tile([1, M], f32)
        nc.vector.tensor_copy(ad[:], adi[:])
        # t1 = a*(d - C + 100) - 100  (== d-C where diag present, else -100)
        t1 = fix.tile([1, M], f32)
        nc.vector.scalar_tensor_tensor(t1[:], dsq[:], 100.0 - C_SHIFT, ad[:],
                                       ALU.add, ALU.mult)
        nc.vector.tensor_scalar_add(t1[:], t1[:], -100.0)
        mmx = fix.tile([1, M], f32)
        nc.vector.tensor_scalar_max(mmx[:], t1[:], 0.0)
        scm = fix.tile([1, M], f32)   # e^{-m}: scale for the off-diag partials
        nc.scalar.activation(scm[:], mmx[:], AF.Exp, scale=-1.0)
        scd = fix.tile([1, M], f32)   # e^{t1-m}: scale for the diag term
        nc.vector.tensor_sub(scd[:], t1[:], mmx[:])
        nc.scalar.activation(scd[:], scd[:], AF.Exp)

        # ---- phase 2: attention loop; out matmuls pipelined one pair back --
        ps_out = ctx.enter_context(tc.tile_pool(name="po", bufs=1,
                                                space="PSUM"))
        po = ps_out.tile([F + 1, M], f32, tag="po", name="po")
        HALVES = (slice(0, 512), slice(512, M))
        pending = None

        def flush_pending(sp):
            j0, lh0, e0, lh1, e1 = pending
            for j, lh, e in ((j0, lh0, e0), (j0 + 1, lh1, e1)):
                st = j == 0
                for half in HALVES:
                    nc.tensor.matmul(po[:, half], lh, e[:, half],
                                     start=st, stop=sp and j == j0 + 1,
                                     skip_group_check=True)

        for q in range(16):
            if q + 4 < 16:
                emit_chunk_mm(q + 4)
            at = at_g[q // 2]
            for hp in range(2):
                j0 = 4 * q + 2 * hp
                l0 = hbc[q][0:F, bass.ts(2 * hp, P)]
                l1 = hbc[q][F : 2 * F, bass.ts(2 * hp + 1, P)]
                ps0 = psP.tile([P, M], f32, tag="ps", name="ps0")
                ps1 = psP.tile([P, M], f32, tag="ps", name="ps1")
                for hi in range(2):
                    nc.tensor.matmul(ps0[:, bass.ts(hi, 512)], l0,
                                     hbc[hi][0:F, :], start=True, stop=True,
                                     tile_position=(0, 0))
                    nc.tensor.matmul(ps1[:, bass.ts(hi, 512)], l1,
                                     hbc[hi][F : 2 * F, :], start=True,
                                     stop=True, tile_position=(64, 0))
                if pending is not None:
                    flush_pending(False)
                pair = []
                for dk, psx in ((0, ps0), (1, ps1)):
                    j = j0 + dk
                    if q < 2:
                        # diag scores (|h_r|^2, up to ~190) would overflow
                        # bf16 after exp; off-diag max is ~99.6 so the clamp
                        # only touches the diagonal, which the mask zeroes
                        # and the tail re-adds exactly
                        nc.vector.tensor_scalar_min(psx[:], psx[:], 150.0)
                    e = ep.tile([P, M], bf16, tag="e", name="e")
                    nc.scalar.activation(e[:], psx[:], AF.Exp, bias=cbias[:])
                    atj = at[:, bass.ts(j % 8, M)]
                    nc.vector.tensor_mul(e[:], e[:], atj)
                    lh = hsbc[q][:, (j % 4) * (F + 1) : (j % 4 + 1) * (F + 1)]
                    pair += [lh, e]
                pending = (j0, *pair)
            if q + 4 < 16:
                emit_chunk_tr(q + 4)
        flush_pending(True)

        # ---- phase 3: merge + normalize ----
        posb = fix.tile([F, M], f32)
        nc.scalar.copy(posb[:], po[0:F, :])
        esum = fix.tile([1, M], f32)
        nc.vector.tensor_copy(esum[:], po[F : F + 1, :])
        den = fix.tile([1, M], f32)
        nc.vector.tensor_mul(den[:], esum[:], scm[:])
        nc.vector.tensor_add(den[:], den[:], scd[:])
        rden = fix.tile([1, M], f32)
        nc.vector.reciprocal_approx_fast(rden[:], den[:])
        alpha = fix.tile([1, M], f32)
        nc.vector.tensor_mul(alpha[:], scm[:], rden[:])
        beta = fix.tile([1, M], f32)
        nc.vector.tensor_mul(beta[:], scd[:], rden[:])

        # broadcast alpha/beta across 64 partitions via K=1 matmul with ones
        ones_row = fix.tile([1, F], f32)
        nc.vector.memset(ones_row[:], 1.0)
        res = fix.tile([F, M], f32)
        res2 = fix.tile([F, M], f32)
        for hi, half in enumerate(HALVES):
            ab = psP.tile([P, 512], f32, tag="ps", name=f"ab{hi}")
            nc.tensor.matmul(ab[0:F, :], ones_row[:], alpha[:, half],
                             start=True, stop=True)
            nc.vector.tensor_mul(res[:, half], posb[:, half], ab[0:F, :])
            bb = psP.tile([P, 512], f32, tag="ps", name=f"bb{hi}")
            nc.tensor.matmul(bb[0:F, :], ones_row[:], beta[:, half],
                             start=True, stop=True)
            nc.vector.tensor_mul(res2[:, half], hto[:, half], bb[0:F, :])
        nc.vector.tensor_add(res[:], res[:], res2[:])
        osb = fix.tile([F, M], f32)
        nc.scalar.activation(osb[:], res[:], AF.Relu)
        nc.sync.dma_start(outT[:], osb[:])


_NC_CACHE = {}


def get_compiled():
    if "nc" not in _NC_CACHE:
        nc = bacc.Bacc("TRN2", target_bir_lowering=False, debug=False,
                       enable_asserts=True, num_devices=NCORES)
        xt = nc.dram_tensor("xt", [D + 1, N], f16, kind="ExternalInput").ap()
        wb = nc.dram_tensor("wb", [D + 1, F], f16, kind="ExternalInput").ap()
        am = nc.dram_tensor("am", [8, P, 8 * M], bf16,
                            kind="ExternalInput").ap()
        adiag = nc.dram_tensor("adiag", [1, M], i32, kind="ExternalInput").ap()
        outT = nc.dram_tensor("outT", [F, M], f32, kind="ExternalOutput").ap()
        with tile.TileContext(nc) as tc:
            nc._tc = tc
            build_kernel(nc, outT, xt, wb, am, adiag)
        nc.compile()
        _NC_CACHE["nc"] = nc
    return _NC_CACHE["nc"]


def make_in_maps(X, A, W, b):
    import ml_dtypes
    X = np.asarray(X, dtype=np.float32)
    A = np.asarray(A)
    if A.dtype != np.int32:
        A = A.astype(np.int32)
    W = np.asarray(W, dtype=np.float32)
    b = np.asarray(b, dtype=np.float32).reshape(1, F)
    wb = np.ascontiguousarray(
        np.concatenate([W, b], axis=0).astype(np.float16))      # [201, 64]
    XT = np.concatenate([X.T, np.ones((1, N), np.float32)],
                        axis=0).astype(np.float16)              # [201, N]
    rng = np.arange(M)
    in_maps = []
    for c in range(NCORES):
        r0 = c * M
        xt_c = np.ascontiguousarray(np.roll(XT, -r0, axis=1))
        blk = np.roll(A[r0 : r0 + M], -r0, axis=1).copy()       # [M, N]
        adiag = blk[rng, rng].reshape(1, M).astype(np.int32)
        blk[rng, rng] = 0            # diag handled exactly by the fp32 tail
        # [8, 128, 8192] bf16 {0,1}: group g lane u holds j = 8g+u
        bits = ((blk.T.reshape(8, 8, P, M) > 0).astype(np.uint16) * 0x3F80)
        am = np.ascontiguousarray(
            bits.transpose(0, 2, 1, 3).reshape(8, P, 8 * M)).view(
                ml_dtypes.bfloat16)
        in_maps.append({"xt": xt_c, "wb": wb, "am": am, "adiag": adiag})
    return in_maps


def kernel(X, A, W, b):
    nc = get_compiled()
    in_maps = make_in_maps(X, A, W, b)
    res = run_bass_kernel_spmd(nc, in_maps, list(range(NCORES)))
    outTs = [res.results[c]["outT"] for c in range(NCORES)]
    return np.ascontiguousarray(np.concatenate(outTs, axis=1).T)


# revision 24
# speedup vs baseline: 2.0906x; 1.0244x over previous
"""Trainium2 Bass kernel for CustomGATConv (dense masked attention GNN layer).

  H = X @ W + b                       [8192, 64]
  S = H @ H.T ; S = where(A>0, S, -1e9)
  out = relu(softmax(S, -1) @ H)      [8192, 64]

Sharding: rows of the score matrix across 8 cores (1024 rows each).
Each core redundantly computes H (tiny) and processes its row block.

v6 design (from v3/v5 traces):
  A HWDGE queue tops out near ~100 GB/s regardless of descriptor size
  (v3: 4KB lines -> 88 GB/s; v5: 16KB lines -> 102 GB/s, engines 73%
  idle either way), so the only DMA levers are fewer bytes and more
  queues.  Mask bit-packing is out: the STT ISA only chains basic arith
  ops, and one monotone scalar op cannot extract two independent bits.
  - masks as uint8 {0,1} (8.4MB/core) in layout [8, 128, 8192] (8KB
    contiguous lines).  Six groups stream on the sync ring from t=0;
    the last two follow xt on the scalar ring.  The e-multiply converts
    u8 inline: DVE (1x) for even j, the otherwise-idle Pool engine for
    odd j.  Zero DMA issues during the loop.
  - the diag clamp shrinks to the 128 columns that can actually hold a
    diagonal element per j<8 tile ([128,128] instead of [128,1024]).
  - H chunks (incl. the partition-64..127 duplicate via a second matmul
    at tile_position col 64 -- no SBUF-to-SBUF dup DMAs) interleave with
    the attention loop; PSUM-read copies split ACT (early chunks) / DVE.
  - out matmuls software-pipelined one pair behind the score matmuls so
    the PE never waits on exp/mask of the current pair.
  - ACT does exp only: 64 x [128,1024] ~ 73us, the steady-state pacer.
"""

import sys
import numpy as np

for _p in ("/opt/trn_rl_repo",):
    if _p not in sys.path:
        sys.path.insert(0, _p)

import concourse.bass as bass
import concourse.tile as tile
from concourse import bacc, mybir
from concourse.bass_utils import run_bass_kernel_spmd
from concourse.masks import make_identity

N = 8192          # nodes
D = 200           # in dim
F = 64            # out dim
NCORES = 8
M = N // NCORES   # 1024 rows per core
P = 128           # partitions
C_SHIFT = 64.0    # global softmax shift for off-diagonal scores

f32 = mybir.dt.float32
bf16 = mybir.dt.bfloat16
f16 = mybir.dt.float16
i16 = mybir.dt.int16
i32 = mybir.dt.int32
AF = mybir.ActivationFunctionType
ALU = mybir.AluOpType


def build_kernel(nc, outT, xt, wb, am, adiag):
    from contextlib import ExitStack

    with ExitStack() as ctx:
        tc = nc._tc
        const = ctx.enter_context(tc.tile_pool(name="const", bufs=1))
        fix = ctx.enter_context(tc.tile_pool(name="fix", bufs=1))
        atp = ctx.enter_context(tc.tile_pool(name="at", bufs=3))
        ep = ctx.enter_context(tc.tile_pool(name="ep", bufs=4))
        psP = ctx.enter_context(tc.tile_pool(name="ps", bufs=3, space="PSUM"))

        hbc = [const.tile([P, 512], f16, tag=f"hb{k}", name=f"hb{k}")
               for k in range(16)]          # H.T fp16, rows 64..127 duplicate
        hsbc = [const.tile([P, 4 * (F + 1)], bf16, tag=f"hs{k}", name=f"hs{k}")
                for k in range(16)]         # per j-tile row-major [H_j | 1]
        hto = const.tile([F, M], f32)       # own-rows H, fp32 (diag path)
        cbias = const.tile([P, 1], f32)     # -C bias for the exp
        ident = const.tile([F, F], f16)     # PE-transpose identity
        xt1c = [const.tile([P, 2048], f16, tag=f"x1{c}", name=f"x1{c}")
                for c in range(4)]
        xt2c = [const.tile([D + 1 - P, 2048], f16, tag=f"x2{c}", name=f"x2{c}")
                for c in range(4)]
        wt1 = const.tile([P, F], f16)
        wt2 = const.tile([D + 1 - P, F], f16)

        # ---- ring issue order.  sync: mask groups 0..5 from t=0 (1MB, 8KB
        # ---- lines each).  scalar: weights, adiag, the four xt column
        # ---- chunks, then mask groups 6..7.  Nothing issues in the loop.
        at_g = []

        def issue_group(g, eng, tag, nb):
            a = atp.tile([P, 8 * M], mybir.dt.uint8, tag=tag,
                         name=f"{tag}{g % nb}")
            eng.dma_start(a[:], am[g])
            at_g.append(a)

        for g in range(6):
            issue_group(g, nc.sync, "at", 3)
        nc.scalar.dma_start(wt1[:], wb[0:P, :])
        nc.scalar.dma_start(wt2[:], wb[P : D + 1, :])
        adi = fix.tile([1, M], i32)
        nc.scalar.dma_start(adi[:], adiag[:])
        for c in range(4):
            nc.scalar.dma_start(xt1c[c][:], xt[0:P, bass.ts(c, 2048)])
            nc.scalar.dma_start(xt2c[c][:], xt[P : D + 1, bass.ts(c, 2048)])
        for g in range(6, 8):
            issue_group(g, nc.scalar, "atb", 2)

        nc.vector.memset(cbias[:], -C_SHIFT)
        make_identity(nc, ident[:])
        for k in range(16):
            h3 = hsbc[k][:].rearrange("p (a b) -> p a b", b=F + 1)
            nc.vector.memset(h3[:, :, F : F + 1], 1.0)

        # ---- phase 1 chunk emitters (chunk c = H.T cols 512c..512c+511) ----
        def emit_chunk_mm(c):
            ps = psP.tile([P, 512], f32, tag="ps", name=f"h{c}")
            c4, s = c // 4, bass.ts(c % 4, 512)
            # rows 0..63 and the 64..127 duplicate, straight from the PE
            nc.tensor.matmul(ps[0:F, :], wt1[:], xt1c[c4][:, s],
                             start=True, stop=False)
            nc.tensor.matmul(ps[0:F, :], wt2[:], xt2c[c4][:, s],
                             start=False, stop=True)
            nc.tensor.matmul(ps[F : 2 * F, :], wt1[:], xt1c[c4][:, s],
                             start=True, stop=False)
            nc.tensor.matmul(ps[F : 2 * F, :], wt2[:], xt2c[c4][:, s],
                             start=False, stop=True)
            # early chunks ride the pre-loop-idle ACT engine, later ones DVE
            if c < 4:
                nc.scalar.copy(hbc[c][:], ps[:])
            else:
                nc.vector.tensor_copy(hbc[c][:], ps[:])
            if c < 2:
                nc.scalar.copy(hto[:, bass.ts(c, 512)], ps[0:F, :])

        def emit_chunk_tr(c):
            trp = psP.tile([P, 4 * F], f16, tag="ps", name=f"t{c}")
            for q in range(4):
                nc.tensor.transpose(trp[:, bass.ts(q, F)],
                                    hbc[c][0:F, bass.ts(q, P)], ident[:])
            h3 = hsbc[c][:].rearrange("p (a b) -> p a b", b=F + 1)
            p3 = trp[:].rearrange("p (a b) -> p a b", b=F)
            if c < 4:
                nc.scalar.copy(h3[:, :, 0:F], p3[:])
            else:
                nc.vector.tensor_copy(h3[:, :, 0:F], p3[:])

        for c in range(4):
            emit_chunk_mm(c)
        for c in range(4):
            emit_chunk_tr(c)

        # ---- diag-score prep: d_r = |h_r|^2, merge scales ----
        htsq = fix.tile([F, M], f32)
        nc.vector.tensor_mul(htsq[:], hto[:], hto[:])
        ones64 = fix.tile([F, 1], f32)
        nc.vector.memset(ones64[:], 1.0)
        dsq = fix.tile([1, M], f32)
        for hi in range(2):
            psd = psP.tile([P, 512], f32, tag="ps", name=f"dsq{hi}")
            nc.tensor.matmul(psd[0:1, :], ones64[:], htsq[:, bass.ts(hi, 512)],
                             start=True, stop=True)
            nc.vector.tensor_copy(dsq[:, bass.ts(hi, 512)], psd[0:1, :])
        ad = fix.tile([1, M], f32)
        nc.vector.tensor_copy(ad[:], adi[:])
        # t1 = a*(d - C + 100) - 100  (== d-C where diag present, else -100)
        t1 = fix.tile([1, M], f32)
        nc.vector.scalar_tensor_tensor(t1[:], dsq[:], 100.0 - C_SHIFT, ad[:],
                                       ALU.add, ALU.mult)
        nc.vector.tensor_scalar_add(t1[:], t1[:], -100.0)
        mmx = fix.tile([1, M], f32)
        nc.vector.tensor_scalar_max(mmx[:], t1[:], 0.0)
        scm = fix.tile([1, M], f32)   # e^{-m}: scale for the off-diag partials
        nc.scalar.activation(scm[:], mmx[:], AF.Exp, scale=-1.0)
        scd = fix.tile([1, M], f32)   # e^{t1-m}: scale for the diag term
        nc.vector.tensor_sub(scd[:], t1[:], mmx[:])
        nc.scalar.activation(scd[:], scd[:], AF.Exp)

        # ---- phase 2: attention loop; out matmuls pipelined one pair back --
        ps_out = ctx.enter_context(tc.tile_pool(name="po", bufs=1,
                                                space="PSUM"))
        po = ps_out.tile([F + 1, M], f32, tag="po", name="po")
        HALVES = (slice(0, 512), slice(512, M))
        pending = None

        def flush_pending(sp):
            j0, lh0, e0, lh1, e1 = pending
            for j, lh, e in ((j0, lh0, e0), (j0 + 1, lh1, e1)):
                st = j == 0
                for half in HALVES:
                    nc.tensor.matmul(po[:, half], lh, e[:, half],
                                     start=st, stop=sp and j == j0 + 1,
                                     skip_group_check=True)

        for q in range(16):
            if q + 4 < 16:
                emit_chunk_mm(q + 4)
            at = at_g[q // 2]
            for hp in range(2):
                j0 = 4 * q + 2 * hp
                l0 = hbc[q][0:F, bass.ts(2 * hp, P)]
                l1 = hbc[q][F : 2 * F, bass.ts(2 * hp + 1, P)]
                ps0 = psP.tile([P, M], f32, tag="ps", name="ps0")
                ps1 = psP.tile([P, M], f32, tag="ps", name="ps1")
                for hi in range(2):
                    nc.tensor.matmul(ps0[:, bass.ts(hi, 512)], l0,
                                     hbc[hi][0:F, :], start=True, stop=True,
                                     tile_position=(0, 0))
                    nc.tensor.matmul(ps1[:, bass.ts(hi, 512)], l1,
                                     hbc[hi][F : 2 * F, :], start=True,
                                     stop=True, tile_position=(64, 0))
                if pending is not None:
                    flush_pending(False)
                pair = []
                for dk, psx in ((0, ps0), (1, ps1)):
                    j = j0 + dk
                    if q < 2:
                        # diag scores (|h_r|^2, up to ~190) would overflow
                        # bf16 after exp; off-diag max is ~99.6 so only the
                        # 128 columns that hold a diagonal element need the
                        # clamp (mask zeroes it; the tail re-adds exactly)
                        dcol = bass.ts(j, P)
                        nc.vector.tensor_scalar_min(psx[:, dcol],
                                                    psx[:, dcol], 150.0)
                    e = ep.tile([P, M], bf16, tag="e", name="e")
                    nc.scalar.activation(e[:], psx[:], AF.Exp, bias=cbias[:])
                    atj = at[:, bass.ts(j % 8, M)]
                    # u8 converts inline; even j on DVE, odd j on Pool
                    if dk == 0:
                        nc.vector.tensor_mul(e[:], e[:], atj)
                    else:
                        nc.gpsimd.tensor_mul(e[:], e[:], atj)
                    lh = hsbc[q][:, (j % 4) * (F + 1) : (j % 4 + 1) * (F + 1)]
                    pair += [lh, e]
                pending = (j0, *pair)
            if q + 4 < 16:
                emit_chunk_tr(q + 4)
        flush_pending(True)

        # ---- phase 3: merge + normalize ----
        posb = fix.tile([F, M], f32)
        nc.scalar.copy(posb[:], po[0:F, :])
        esum = fix.tile([1, M], f32)
        nc.vector.tensor_copy(esum[:], po[F : F + 1, :])
        den = fix.tile([1, M], f32)
        nc.vector.tensor_mul(den[:], esum[:], scm[:])
        nc.vector.tensor_add(den[:], den[:], scd[:])
        rden = fix.tile([1, M], f32)
        nc.vector.reciprocal_approx_fast(rden[:], den[:])
        alpha = fix.tile([1, M], f32)
        nc.vector.tensor_mul(alpha[:], scm[:], rden[:])
        beta = fix.tile([1, M], f32)
        nc.vector.tensor_mul(beta[:], scd[:], rden[:])

        # broadcast alpha/beta across 64 partitions via K=1 matmul with ones
        ones_row = fix.tile([1, F], f32)
        nc.vector.memset(ones_row[:], 1.0)
        res = fix.tile([F, M], f32)
        res2 = fix.tile([F, M], f32)
        for hi, half in enumerate(HALVES):
            ab = psP.tile([P, 512], f32, tag="ps", name=f"ab{hi}")
            nc.tensor.matmul(ab[0:F, :], ones_row[:], alpha[:, half],
                             start=True, stop=True)
            nc.vector.tensor_mul(res[:, half], posb[:, half], ab[0:F, :])
            bb = psP.tile([P, 512], f32, tag="ps", name=f"bb{hi}")
            nc.tensor.matmul(bb[0:F, :], ones_row[:], beta[:, half],
                             start=True, stop=True)
            nc.vector.tensor_mul(res2[:, half], hto[:, half], bb[0:F, :])
        nc.vector.tensor_add(res[:], res[:], res2[:])
        osb = fix.tile([F, M], f32)
        nc.scalar.activation(osb[:], res[:], AF.Relu)
        nc.sync.dma_start(outT[:], osb[:])


_NC_CACHE = {}


def get_compiled():
    if "nc" not in _NC_CACHE:
        nc = bacc.Bacc("TRN2", target_bir_lowering=False, debug=False,
                       enable_asserts=True, num_devices=NCORES)
        xt = nc.dram_tensor("xt", [D + 1, N], f16, kind="ExternalInput").ap()
        wb = nc.dram_tensor("wb", [D + 1, F], f16, kind="ExternalInput").ap()
        am = nc.dram_tensor("am", [8, P, 8 * M], mybir.dt.uint8,
                            kind="ExternalInput").ap()
        adiag = nc.dram_tensor("adiag", [1, M], i32, kind="ExternalInput").ap()
        outT = nc.dram_tensor("outT", [F, M], f32, kind="ExternalOutput").ap()
        with tile.TileContext(nc) as tc:
            nc._tc = tc
            build_kernel(nc, outT, xt, wb, am, adiag)
        nc.compile()
        _NC_CACHE["nc"] = nc
    return _NC_CACHE["nc"]


def make_in_maps(X, A, W, b):
    import ml_dtypes
    X = np.asarray(X, dtype=np.float32)
    A = np.asarray(A)
    if A.dtype != np.int32:
        A = A.astype(np.int32)
    W = np.asarray(W, dtype=np.float32)
    b = np.asarray(b, dtype=np.float32).reshape(1, F)
    wb = np.ascontiguousarray(
        np.concatenate([W, b], axis=0).astype(np.float16))      # [201, 64]
    XT = np.concatenate([X.T, np.ones((1, N), np.float32)],
                        axis=0).astype(np.float16)              # [201, N]
    rng = np.arange(M)
    in_maps = []
    for c in range(NCORES):
        r0 = c * M
        xt_c = np.ascontiguousarray(np.roll(XT, -r0, axis=1))
        blk = np.roll(A[r0 : r0 + M], -r0, axis=1).copy()       # [M, N]
        adiag = blk[rng, rng].reshape(1, M).astype(np.int32)
        blk[rng, rng] = 0            # diag handled exactly by the fp32 tail
        # [8, 128, 8192] u8 {0,1}: group g lane u holds j = 8g+u
        bits = (blk.T.reshape(8, 8, P, M) > 0).astype(np.uint8)
        am = np.ascontiguousarray(bits.transpose(0, 2, 1, 3)
                                  .reshape(8, P, 8 * M))
        in_maps.append({"xt": xt_c, "wb": wb, "am": am, "adiag": adiag})
    return in_maps


def kernel(X, A, W, b):
    nc = get_compiled()
    in_maps = make_in_maps(X, A, W, b)
    res = run_bass_kernel_spmd(nc, in_maps, list(range(NCORES)))
    outTs = [res.results[c]["outT"] for c in range(NCORES)]
    return np.ascontiguousarray(np.concatenate(outTs, axis=1).T)


# revision 31
# speedup vs baseline: 2.2359x; 1.0695x over previous
"""Trainium2 Bass kernel for CustomGATConv (dense masked attention GNN layer).

  H = X @ W + b                       [8192, 64]
  S = H @ H.T ; S = where(A>0, S, -1e9)
  out = relu(softmax(S, -1) @ H)      [8192, 64]

Sharding: rows of the score matrix across 8 cores (1024 rows each).
Each core redundantly computes H (tiny) and processes its row block.

v7 design (from v3/v5/v6 traces):
  A HWDGE queue tops out near ~100 GB/s regardless of descriptor size,
  and non-16-bit masks wreck the e-multiply (u8 drops DVE to 1x and
  Pool's software mult costs 2.5us/tile -> v6's loop ran at mask-mult
  pace).  Bit-packing is out (STT chains only basic arith ops; one
  monotone scalar op can't extract two bits).  So masks stay bf16 and
  the DMA plan spreads them over THREE queues:
  - [16, 128, 4096] bf16 groups (0.5MB, 8KB lines, 4 j-tiles each):
    10 on the sync ring from t=0, 4 on the scalar ring after xt, and 2
    on the gpsimd software-DGE ring (throughput experiment).
  - e-multiply: one DVE 2x tensor_mul per j (0.65us).
  - the diag clamp shrinks to the 128 columns that can actually hold a
    diagonal element per j<8 tile ([128,128] instead of [128,1024]).
  - H chunks (incl. the partition-64..127 duplicate via a second matmul
    at tile_position col 64 -- no SBUF-to-SBUF dup DMAs) interleave with
    the attention loop; PSUM-read copies split ACT (early chunks) / DVE.
  - out matmuls software-pipelined one pair behind the score matmuls so
    the PE never waits on exp/mask of the current pair.
  - ACT does exp only: 64 x [128,1024] ~ 73us, the steady-state pacer.
"""

import sys
import numpy as np

for _p in ("/opt/trn_rl_repo",):
    if _p not in sys.path:
        sys.path.insert(0, _p)

import concourse.bass as bass
import concourse.tile as tile
from concourse import bacc, mybir
from concourse.bass_utils import run_bass_kernel_spmd
from concourse.masks import make_identity

N = 8192          # nodes
D = 200           # in dim
F = 64            # out dim
NCORES = 8
M = N // NCORES   # 1024 rows per core
P = 128           # partitions
C_SHIFT = 64.0    # global softmax shift for off-diagonal scores

f32 = mybir.dt.float32
bf16 = mybir.dt.bfloat16
f16 = mybir.dt.float16
i16 = mybir.dt.int16
i32 = mybir.dt.int32
AF = mybir.ActivationFunctionType
ALU = mybir.AluOpType


def build_kernel(nc, outT, xt, wb, am, adiag):
    from contextlib import ExitStack

    with ExitStack() as ctx:
        tc = nc._tc
        const = ctx.enter_context(tc.tile_pool(name="const", bufs=1))
        fix = ctx.enter_context(tc.tile_pool(name="fix", bufs=1))
        atp = ctx.enter_context(tc.tile_pool(name="at", bufs=3))
        ep = ctx.enter_context(tc.tile_pool(name="ep", bufs=4))
        psP = ctx.enter_context(tc.tile_pool(name="ps", bufs=3, space="PSUM"))

        hbc = [const.tile([P, 512], f16, tag=f"hb{k}", name=f"hb{k}")
               for k in range(16)]          # H.T fp16, rows 64..127 duplicate
        hsbc = [const.tile([P, 4 * (F + 1)], bf16, tag=f"hs{k}", name=f"hs{k}")
                for k in range(16)]         # per j-tile row-major [H_j | 1]
        hto = const.tile([F, M], f32)       # own-rows H, fp32 (diag path)
        cbias = const.tile([P, 1], f32)     # -C bias for the exp
        ident = const.tile([F, F], f16)     # PE-transpose identity
        xt1c = [const.tile([P, 2048], f16, tag=f"x1{c}", name=f"x1{c}")
                for c in range(4)]
        xt2c = [const.tile([D + 1 - P, 2048], f16, tag=f"x2{c}", name=f"x2{c}")
                for c in range(4)]
        wt1 = const.tile([P, F], f16)
        wt2 = const.tile([D + 1 - P, F], f16)

        # ---- ring issue order.  sync: mask groups 0..9 from t=0 (0.5MB,
        # ---- 8KB lines, 4 j-tiles each).  scalar: weights, adiag, the
        # ---- four xt column chunks, then groups 10..13.  gpsimd SWDGE:
        # ---- groups 14..15.  Nothing issues during the loop.
        at_g = []

        def issue_group(g, eng, tag, nb):
            a = atp.tile([P, 4 * M], bf16, tag=tag, name=f"{tag}{g % nb}")
            eng.dma_start(a[:], am[g])
            at_g.append(a)

        for g in range(10):
            issue_group(g, nc.sync, "at", 3)
        nc.scalar.dma_start(wt1[:], wb[0:P, :])
        nc.scalar.dma_start(wt2[:], wb[P : D + 1, :])
        adi = fix.tile([1, M], i32)
        nc.scalar.dma_start(adi[:], adiag[:])
        for c in range(4):
            nc.scalar.dma_start(xt1c[c][:], xt[0:P, bass.ts(c, 2048)])
            nc.scalar.dma_start(xt2c[c][:], xt[P : D + 1, bass.ts(c, 2048)])
        for g in range(10, 14):
            issue_group(g, nc.scalar, "atb", 2)
        for g in range(14, 16):
            issue_group(g, nc.gpsimd, "atc", 2)

        nc.vector.memset(cbias[:], -C_SHIFT)
        make_identity(nc, ident[:])
        for k in range(16):
            h3 = hsbc[k][:].rearrange("p (a b) -> p a b", b=F + 1)
            nc.vector.memset(h3[:, :, F : F + 1], 1.0)

        # ---- phase 1 chunk emitters (chunk c = H.T cols 512c..512c+511) ----
        def emit_chunk_mm(c):
            ps = psP.tile([P, 512], f32, tag="ps", name=f"h{c}")
            c4, s = c // 4, bass.ts(c % 4, 512)
            # rows 0..63 and the 64..127 duplicate, straight from the PE
            nc.tensor.matmul(ps[0:F, :], wt1[:], xt1c[c4][:, s],
                             start=True, stop=False)
            nc.tensor.matmul(ps[0:F, :], wt2[:], xt2c[c4][:, s],
                             start=False, stop=True)
            nc.tensor.matmul(ps[F : 2 * F, :], wt1[:], xt1c[c4][:, s],
                             start=True, stop=False)
            nc.tensor.matmul(ps[F : 2 * F, :], wt2[:], xt2c[c4][:, s],
                             start=False, stop=True)
            # early chunks ride the pre-loop-idle ACT engine, later ones DVE
            if c < 4:
                nc.scalar.copy(hbc[c][:], ps[:])
            else:
                nc.vector.tensor_copy(hbc[c][:], ps[:])
            if c < 2:
                nc.scalar.copy(hto[:, bass.ts(c, 512)], ps[0:F, :])

        def emit_chunk_tr(c):
            trp = psP.tile([P, 4 * F], f16, tag="ps", name=f"t{c}")
            for q in range(4):
                nc.tensor.transpose(trp[:, bass.ts(q, F)],
                                    hbc[c][0:F, bass.ts(q, P)], ident[:])
            h3 = hsbc[c][:].rearrange("p (a b) -> p a b", b=F + 1)
            p3 = trp[:].rearrange("p (a b) -> p a b", b=F)
            if c < 4:
                nc.scalar.copy(h3[:, :, 0:F], p3[:])
            else:
                nc.vector.tensor_copy(h3[:, :, 0:F], p3[:])

        for c in range(4):
            emit_chunk_mm(c)
        for c in range(4):
            emit_chunk_tr(c)

        # ---- diag-score prep: d_r = |h_r|^2, merge scales ----
        htsq = fix.tile([F, M], f32)
        nc.vector.tensor_mul(htsq[:], hto[:], hto[:])
        ones64 = fix.tile([F, 1], f32)
        nc.vector.memset(ones64[:], 1.0)
        dsq = fix.tile([1, M], f32)
        for hi in range(2):
            psd = psP.tile([P, 512], f32, tag="ps", name=f"dsq{hi}")
            nc.tensor.matmul(psd[0:1, :], ones64[:], htsq[:, bass.ts(hi, 512)],
                             start=True, stop=True)
            nc.vector.tensor_copy(dsq[:, bass.ts(hi, 512)], psd[0:1, :])
        ad = fix.tile([1, M], f32)
        nc.vector.tensor_copy(ad[:], adi[:])
        # t1 = a*(d - C + 100) - 100  (== d-C where diag present, else -100)
        t1 = fix.tile([1, M], f32)
        nc.vector.scalar_tensor_tensor(t1[:], dsq[:], 100.0 - C_SHIFT, ad[:],
                                       ALU.add, ALU.mult)
        nc.vector.tensor_scalar_add(t1[:], t1[:], -100.0)
        mmx = fix.tile([1, M], f32)
        nc.vector.tensor_scalar_max(mmx[:], t1[:], 0.0)
        scm = fix.tile([1, M], f32)   # e^{-m}: scale for the off-diag partials
        nc.scalar.activation(scm[:], mmx[:], AF.Exp, scale=-1.0)
        scd = fix.tile([1, M], f32)   # e^{t1-m}: scale for the diag term
        nc.vector.tensor_sub(scd[:], t1[:], mmx[:])
        nc.scalar.activation(scd[:], scd[:], AF.Exp)

        # ---- phase 2: attention loop; out matmuls pipelined one pair back --
        ps_out = ctx.enter_context(tc.tile_pool(name="po", bufs=1,
                                                space="PSUM"))
        po = ps_out.tile([F + 1, M], f32, tag="po", name="po")
        HALVES = (slice(0, 512), slice(512, M))
        pending = None

        def flush_pending(sp):
            j0, lh0, e0, lh1, e1 = pending
            for j, lh, e in ((j0, lh0, e0), (j0 + 1, lh1, e1)):
                st = j == 0
                for half in HALVES:
                    nc.tensor.matmul(po[:, half], lh, e[:, half],
                                     start=st, stop=sp and j == j0 + 1,
                                     skip_group_check=True)

        for q in range(16):
            if q + 4 < 16:
                emit_chunk_mm(q + 4)
            at = at_g[q]
            for hp in range(2):
                j0 = 4 * q + 2 * hp
                l0 = hbc[q][0:F, bass.ts(2 * hp, P)]
                l1 = hbc[q][F : 2 * F, bass.ts(2 * hp + 1, P)]
                ps0 = psP.tile([P, M], f32, tag="ps", name="ps0")
                ps1 = psP.tile([P, M], f32, tag="ps", name="ps1")
                for hi in range(2):
                    nc.tensor.matmul(ps0[:, bass.ts(hi, 512)], l0,
                                     hbc[hi][0:F, :], start=True, stop=True,
                                     tile_position=(0, 0))
                    nc.tensor.matmul(ps1[:, bass.ts(hi, 512)], l1,
                                     hbc[hi][F : 2 * F, :], start=True,
                                     stop=True, tile_position=(64, 0))
                if pending is not None:
                    flush_pending(False)
                pair = []
                for dk, psx in ((0, ps0), (1, ps1)):
                    j = j0 + dk
                    if q < 2:
                        # diag scores (|h_r|^2, up to ~190) would overflow
                        # bf16 after exp; off-diag max is ~99.6 so only the
                        # 128 columns that hold a diagonal element need the
                        # clamp (mask zeroes it; the tail re-adds exactly)
                        dcol = bass.ts(j, P)
                        nc.vector.tensor_scalar_min(psx[:, dcol],
                                                    psx[:, dcol], 150.0)
                    e = ep.tile([P, M], bf16, tag="e", name="e")
                    nc.scalar.activation(e[:], psx[:], AF.Exp, bias=cbias[:])
                    nc.vector.tensor_mul(e[:], e[:], at[:, bass.ts(j % 4, M)])
                    lh = hsbc[q][:, (j % 4) * (F + 1) : (j % 4 + 1) * (F + 1)]
                    pair += [lh, e]
                pending = (j0, *pair)
            if q + 4 < 16:
                emit_chunk_tr(q + 4)
        flush_pending(True)

        # ---- phase 3: merge + normalize ----
        posb = fix.tile([F, M], f32)
        nc.scalar.copy(posb[:], po[0:F, :])
        esum = fix.tile([1, M], f32)
        nc.vector.tensor_copy(esum[:], po[F : F + 1, :])
        den = fix.tile([1, M], f32)
        nc.vector.tensor_mul(den[:], esum[:], scm[:])
        nc.vector.tensor_add(den[:], den[:], scd[:])
        rden = fix.tile([1, M], f32)
        nc.vector.reciprocal_approx_fast(rden[:], den[:])
        alpha = fix.tile([1, M], f32)
        nc.vector.tensor_mul(alpha[:], scm[:], rden[:])
        beta = fix.tile([1, M], f32)
        nc.vector.tensor_mul(beta[:], scd[:], rden[:])

        # broadcast alpha/beta across 64 partitions via K=1 matmul with ones.
        # posb and htsq (dead by now) serve as the result scratch tiles.
        ones_row = fix.tile([1, F], f32)
        nc.vector.memset(ones_row[:], 1.0)
        for hi, half in enumerate(HALVES):
            ab = psP.tile([P, 512], f32, tag="ps", name=f"ab{hi}")
            nc.tensor.matmul(ab[0:F, :], ones_row[:], alpha[:, half],
                             start=True, stop=True)
            nc.vector.tensor_mul(posb[:, half], posb[:, half], ab[0:F, :])
            bb = psP.tile([P, 512], f32, tag="ps", name=f"bb{hi}")
            nc.tensor.matmul(bb[0:F, :], ones_row[:], beta[:, half],
                             start=True, stop=True)
            nc.vector.tensor_mul(htsq[:, half], hto[:, half], bb[0:F, :])
        nc.vector.tensor_add(posb[:], posb[:], htsq[:])
        nc.scalar.activation(posb[:], posb[:], AF.Relu)
        nc.sync.dma_start(outT[:], posb[:])


_NC_CACHE = {}


def get_compiled():
    if "nc" not in _NC_CACHE:
        nc = bacc.Bacc("TRN2", target_bir_lowering=False, debug=False,
                       enable_asserts=True, num_devices=NCORES)
        xt = nc.dram_tensor("xt", [D + 1, N], f16, kind="ExternalInput").ap()
        wb = nc.dram_tensor("wb", [D + 1, F], f16, kind="ExternalInput").ap()
        am = nc.dram_tensor("am", [16, P, 4 * M], bf16,
                            kind="ExternalInput").ap()
        adiag = nc.dram_tensor("adiag", [1, M], i32, kind="ExternalInput").ap()
        outT = nc.dram_tensor("outT", [F, M], f32, kind="ExternalOutput").ap()
        with tile.TileContext(nc) as tc:
            nc._tc = tc
            build_kernel(nc, outT, xt, wb, am, adiag)
        nc.compile()
        _NC_CACHE["nc"] = nc
    return _NC_CACHE["nc"]


def make_in_maps(X, A, W, b):
    import ml_dtypes
    X = np.asarray(X, dtype=np.float32)
    A = np.asarray(A)
    if A.dtype != np.int32:
        A = A.astype(np.int32)
    W = np.asarray(W, dtype=np.float32)
    b = np.asarray(b, dtype=np.float32).reshape(1, F)
    wb = np.ascontiguousarray(
        np.concatenate([W, b], axis=0).astype(np.float16))      # [201, 64]
    XT = np.concatenate([X.T, np.ones((1, N), np.float32)],
                        axis=0).astype(np.float16)              # [201, N]
    rng = np.arange(M)
    in_maps = []
    for c in range(NCORES):
        r0 = c * M
        xt_c = np.ascontiguousarray(np.roll(XT, -r0, axis=1))
        blk = np.roll(A[r0 : r0 + M], -r0, axis=1).copy()       # [M, N]
        adiag = blk[rng, rng].reshape(1, M).astype(np.int32)
        blk[rng, rng] = 0            # diag handled exactly by the fp32 tail
        # [16, 128, 4096] bf16 {0,1}: group g lane u holds j = 4g+u
        bits = ((blk.T.reshape(16, 4, P, M) > 0).astype(np.uint16) * 0x3F80)
        am = np.ascontiguousarray(bits.transpose(0, 2, 1, 3)
                                  .reshape(16, P, 4 * M)).view(
                                      ml_dtypes.bfloat16)
        in_maps.append({"xt": xt_c, "wb": wb, "am": am, "adiag": adiag})
    return in_maps


def kernel(X, A, W, b):
    nc = get_compiled()
    in_maps = make_in_maps(X, A, W, b)
    res = run_bass_kernel_spmd(nc, in_maps, list(range(NCORES)))
    outTs = [res.results[c]["outT"] for c in range(NCORES)]
    return np.ascontiguousarray(np.concatenate(outTs, axis=1).T)


# revision 38
# speedup vs baseline: 2.5041x; 1.1200x over previous
"""Trainium2 Bass kernel for CustomGATConv (dense masked attention GNN layer).

  H = X @ W + b                       [8192, 64]
  S = H @ H.T ; S = where(A>0, S, -1e9)
  out = relu(softmax(S, -1) @ H)      [8192, 64]

Sharding: rows of the score matrix across 8 cores (1024 rows each).
Each core redundantly computes H (tiny) and processes its row block.

v7 design (from v3/v5/v6 traces):
  A HWDGE queue tops out near ~100 GB/s regardless of descriptor size,
  and non-16-bit masks wreck the e-multiply (u8 drops DVE to 1x and
  Pool's software mult costs 2.5us/tile -> v6's loop ran at mask-mult
  pace).  Bit-packing is out (STT chains only basic arith ops; one
  monotone scalar op can't extract two bits).  So masks stay bf16 and
  the DMA plan spreads them over the queues whose issue cost is free:
  v7 measured the gpsimd software-DGE at 127 GB/s with issue running on
  the idle gpsimd engine (~12.5us per 128-descriptor DMA), while
  scalar-ring issues burn ~4us EACH on the ACT engine (62us of exp time
  lost).  So:
  - masks [8, 128, 8192] bf16 (2MB groups, 16KB lines, 8 j-tiles each):
    groups 0,1,2,3,5,7 on the software-DGE ring (~160 GB/s sustained),
    groups 4,6 on the sync ring after the xt chunks.  The scalar ring
    issues only wb/adiag.  Zero DMA issues during the loop.
  - e-multiply: one DVE 2x tensor_mul per j (0.65us).
  - the diag clamp shrinks to the 128 columns that can actually hold a
    diagonal element per j<8 tile ([128,128] instead of [128,1024]).
  - H chunks (incl. the partition-64..127 duplicate via a second matmul
    at tile_position col 64 -- no SBUF-to-SBUF dup DMAs) interleave with
    the attention loop; PSUM-read copies split ACT (early chunks) / DVE.
  - out matmuls software-pipelined one pair behind the score matmuls so
    the PE never waits on exp/mask of the current pair.
  - ACT does exp only: 64 x [128,1024] ~ 73us, the steady-state pacer.
"""

import sys
import numpy as np

for _p in ("/opt/trn_rl_repo",):
    if _p not in sys.path:
        sys.path.insert(0, _p)

import concourse.bass as bass
import concourse.tile as tile
from concourse import bacc, mybir
from concourse.bass_utils import run_bass_kernel_spmd
from concourse.masks import make_identity

N = 8192          # nodes
D = 200           # in dim
F = 64            # out dim
NCORES = 8
M = N // NCORES   # 1024 rows per core
P = 128           # partitions
C_SHIFT = 64.0    # global softmax shift for off-diagonal scores

f32 = mybir.dt.float32
bf16 = mybir.dt.bfloat16
f16 = mybir.dt.float16
i16 = mybir.dt.int16
i32 = mybir.dt.int32
AF = mybir.ActivationFunctionType
ALU = mybir.AluOpType


def build_kernel(nc, outT, xt, wb, am, adiag):
    from contextlib import ExitStack

    with ExitStack() as ctx:
        tc = nc._tc
        const = ctx.enter_context(tc.tile_pool(name="const", bufs=1))
        fix = ctx.enter_context(tc.tile_pool(name="fix", bufs=1))
        atp = ctx.enter_context(tc.tile_pool(name="at", bufs=2))
        ep = ctx.enter_context(tc.tile_pool(name="ep", bufs=4))
        psP = ctx.enter_context(tc.tile_pool(name="ps", bufs=3, space="PSUM"))

        hbc = [const.tile([P, 512], f16, tag=f"hb{k}", name=f"hb{k}")
               for k in range(16)]          # H.T fp16, rows 64..127 duplicate
        hsbc = [const.tile([P, 4 * (F + 1)], bf16, tag=f"hs{k}", name=f"hs{k}")
                for k in range(16)]         # per j-tile row-major [H_j | 1]
        hto = const.tile([F, M], f32)       # own-rows H, fp32 (diag path)
        cbias = const.tile([P, 1], f32)     # -C bias for the exp
        ident = const.tile([F, F], f16)     # PE-transpose identity
        xt1c = [const.tile([P, 2048], f16, tag=f"x1{c}", name=f"x1{c}")
                for c in range(4)]
        xt2c = [const.tile([D + 1 - P, 2048], f16, tag=f"x2{c}", name=f"x2{c}")
                for c in range(4)]
        wt1 = const.tile([P, F], f16)
        wt2 = const.tile([D + 1 - P, F], f16)

        # ---- ring issue order.  gpsimd SWDGE: mask groups 0,1,2,3,5,7
        # ---- (2MB, 16KB lines, 8 j-tiles each) from t=0.  sync: the four
        # ---- xt column-chunk pairs, then groups 4, 6 and outT.  scalar:
        # ---- only wb/adiag (its issues run ON the ACT engine).
        at_tiles = {}

        def issue_group(g, eng, tag, nb, i):
            a = atp.tile([P, 8 * M], bf16, tag=tag, name=f"{tag}{i % nb}")
            eng.dma_start(a[:], am[g])
            at_tiles[g] = a

        for i, g in enumerate((0, 1, 2, 3, 5, 7)):
            issue_group(g, nc.gpsimd, "at", 2, i)
        nc.scalar.dma_start(wt1[:], wb[0:P, :])
        nc.scalar.dma_start(wt2[:], wb[P : D + 1, :])
        adi = fix.tile([1, M], i32)
        nc.scalar.dma_start(adi[:], adiag[:])
        for c in range(4):
            nc.sync.dma_start(xt1c[c][:], xt[0:P, bass.ts(c, 2048)])
            nc.sync.dma_start(xt2c[c][:], xt[P : D + 1, bass.ts(c, 2048)])
        for i, g in enumerate((4, 6)):
            issue_group(g, nc.sync, "atb", 2, i)

        nc.vector.memset(cbias[:], -C_SHIFT)
        make_identity(nc, ident[:])
        for k in range(16):
            h3 = hsbc[k][:].rearrange("p (a b) -> p a b", b=F + 1)
            nc.vector.memset(h3[:, :, F : F + 1], 1.0)

        # ---- phase 1 chunk emitters (chunk c = H.T cols 512c..512c+511) ----
        def emit_chunk_mm(c):
            ps = psP.tile([P, 512], f32, tag="ps", name=f"h{c}")
            c4, s = c // 4, bass.ts(c % 4, 512)
            # rows 0..63 and the 64..127 duplicate, straight from the PE
            nc.tensor.matmul(ps[0:F, :], wt1[:], xt1c[c4][:, s],
                             start=True, stop=False)
            nc.tensor.matmul(ps[0:F, :], wt2[:], xt2c[c4][:, s],
                             start=False, stop=True)
            nc.tensor.matmul(ps[F : 2 * F, :], wt1[:], xt1c[c4][:, s],
                             start=True, stop=False)
            nc.tensor.matmul(ps[F : 2 * F, :], wt2[:], xt2c[c4][:, s],
                             start=False, stop=True)
            # early chunks ride the pre-loop-idle ACT engine, later ones DVE
            if c < 4:
                nc.scalar.copy(hbc[c][:], ps[:])
            else:
                nc.vector.tensor_copy(hbc[c][:], ps[:])
            if c < 2:
                nc.scalar.copy(hto[:, bass.ts(c, 512)], ps[0:F, :])

        def emit_chunk_tr(c):
            trp = psP.tile([P, 4 * F], f16, tag="ps", name=f"t{c}")
            for q in range(4):
                nc.tensor.transpose(trp[:, bass.ts(q, F)],
                                    hbc[c][0:F, bass.ts(q, P)], ident[:])
            h3 = hsbc[c][:].rearrange("p (a b) -> p a b", b=F + 1)
            p3 = trp[:].rearrange("p (a b) -> p a b", b=F)
            if c < 4:
                nc.scalar.copy(h3[:, :, 0:F], p3[:])
            else:
                nc.vector.tensor_copy(h3[:, :, 0:F], p3[:])

        for c in range(4):
            emit_chunk_mm(c)
        for c in range(4):
            emit_chunk_tr(c)

        # ---- diag-score prep: d_r = |h_r|^2, merge scales ----
        htsq = fix.tile([F, M], f32)
        nc.vector.tensor_mul(htsq[:], hto[:], hto[:])
        ones64 = fix.tile([F, 1], f32)
        nc.vector.memset(ones64[:], 1.0)
        dsq = fix.tile([1, M], f32)
        for hi in range(2):
            psd = psP.tile([P, 512], f32, tag="ps", name=f"dsq{hi}")
            nc.tensor.matmul(psd[0:1, :], ones64[:], htsq[:, bass.ts(hi, 512)],
                             start=True, stop=True)
            nc.vector.tensor_copy(dsq[:, bass.ts(hi, 512)], psd[0:1, :])
        ad = fix.tile([1, M], f32)
        nc.vector.tensor_copy(ad[:], adi[:])
        # t1 = a*(d - C + 100) - 100  (== d-C where diag present, else -100)
        t1 = fix.tile([1, M], f32)
        nc.vector.scalar_tensor_tensor(t1[:], dsq[:], 100.0 - C_SHIFT, ad[:],
                                       ALU.add, ALU.mult)
        nc.vector.tensor_scalar_add(t1[:], t1[:], -100.0)
        mmx = fix.tile([1, M], f32)
        nc.vector.tensor_scalar_max(mmx[:], t1[:], 0.0)
        scm = fix.tile([1, M], f32)   # e^{-m}: scale for the off-diag partials
        nc.scalar.activation(scm[:], mmx[:], AF.Exp, scale=-1.0)
        scd = fix.tile([1, M], f32)   # e^{t1-m}: scale for the diag term
        nc.vector.tensor_sub(scd[:], t1[:], mmx[:])
        nc.scalar.activation(scd[:], scd[:], AF.Exp)

        # ---- phase 2: attention loop; out matmuls pipelined one pair back --
        ps_out = ctx.enter_context(tc.tile_pool(name="po", bufs=1,
                                                space="PSUM"))
        po = ps_out.tile([F + 1, M], f32, tag="po", name="po")
        HALVES = (slice(0, 512), slice(512, M))
        pending = None

        def flush_pending(sp):
            j0, lh0, e0, lh1, e1 = pending
            for j, lh, e in ((j0, lh0, e0), (j0 + 1, lh1, e1)):
                st = j == 0
                for half in HALVES:
                    nc.tensor.matmul(po[:, half], lh, e[:, half],
                                     start=st, stop=sp and j == j0 + 1,
                                     skip_group_check=True)

        for q in range(16):
            if q + 4 < 16:
                emit_chunk_mm(q + 4)
            at = at_tiles[q // 2]
            for hp in range(2):
                j0 = 4 * q + 2 * hp
                l0 = hbc[q][0:F, bass.ts(2 * hp, P)]
                l1 = hbc[q][F : 2 * F, bass.ts(2 * hp + 1, P)]
                ps0 = psP.tile([P, M], f32, tag="ps", name="ps0")
                ps1 = psP.tile([P, M], f32, tag="ps", name="ps1")
                for hi in range(2):
                    nc.tensor.matmul(ps0[:, bass.ts(hi, 512)], l0,
                                     hbc[hi][0:F, :], start=True, stop=True,
                                     tile_position=(0, 0))
                    nc.tensor.matmul(ps1[:, bass.ts(hi, 512)], l1,
                                     hbc[hi][F : 2 * F, :], start=True,
                                     stop=True, tile_position=(64, 0))
                if pending is not None:
                    flush_pending(False)
                pair = []
                for dk, psx in ((0, ps0), (1, ps1)):
                    j = j0 + dk
                    if q < 2:
                        # diag scores (|h_r|^2, up to ~190) would overflow
                        # bf16 after exp; off-diag max is ~99.6 so only the
                        # 128 columns that hold a diagonal element need the
                        # clamp (mask zeroes it; the tail re-adds exactly)
                        dcol = bass.ts(j, P)
                        nc.vector.tensor_scalar_min(psx[:, dcol],
                                                    psx[:, dcol], 150.0)
                    e = ep.tile([P, M], bf16, tag="e", name="e")
                    nc.scalar.activation(e[:], psx[:], AF.Exp, bias=cbias[:])
                    nc.vector.tensor_mul(e[:], e[:], at[:, bass.ts(j % 8, M)])
                    lh = hsbc[q][:, (j % 4) * (F + 1) : (j % 4 + 1) * (F + 1)]
                    pair += [lh, e]
                pending = (j0, *pair)
            if q + 4 < 16:
                emit_chunk_tr(q + 4)
        flush_pending(True)

        # ---- phase 3: merge + normalize ----
        posb = fix.tile([F, M], f32)
        nc.scalar.copy(posb[:], po[0:F, :])
        esum = fix.tile([1, M], f32)
        nc.vector.tensor_copy(esum[:], po[F : F + 1, :])
        den = fix.tile([1, M], f32)
        nc.vector.tensor_mul(den[:], esum[:], scm[:])
        nc.vector.tensor_add(den[:], den[:], scd[:])
        rden = fix.tile([1, M], f32)
        nc.vector.reciprocal_approx_fast(rden[:], den[:])
        alpha = fix.tile([1, M], f32)
        nc.vector.tensor_mul(alpha[:], scm[:], rden[:])
        beta = fix.tile([1, M], f32)
        nc.vector.tensor_mul(beta[:], scd[:], rden[:])

        # broadcast alpha/beta across 64 partitions via K=1 matmul with ones.
        # posb and htsq (dead by now) serve as the result scratch tiles.
        ones_row = fix.tile([1, F], f32)
        nc.vector.memset(ones_row[:], 1.0)
        for hi, half in enumerate(HALVES):
            ab = psP.tile([P, 512], f32, tag="ps", name=f"ab{hi}")
            nc.tensor.matmul(ab[0:F, :], ones_row[:], alpha[:, half],
                             start=True, stop=True)
            nc.vector.tensor_mul(posb[:, half], posb[:, half], ab[0:F, :])
            bb = psP.tile([P, 512], f32, tag="ps", name=f"bb{hi}")
            nc.tensor.matmul(bb[0:F, :], ones_row[:], beta[:, half],
                             start=True, stop=True)
            nc.vector.tensor_mul(htsq[:, half], hto[:, half], bb[0:F, :])
        nc.vector.tensor_add(posb[:], posb[:], htsq[:])
        nc.scalar.activation(posb[:], posb[:], AF.Relu)
        nc.sync.dma_start(outT[:], posb[:])


_NC_CACHE = {}


def get_compiled():
    if "nc" not in _NC_CACHE:
        nc = bacc.Bacc("TRN2", target_bir_lowering=False, debug=False,
                       enable_asserts=True, num_devices=NCORES)
        xt = nc.dram_tensor("xt", [D + 1, N], f16, kind="ExternalInput").ap()
        wb = nc.dram_tensor("wb", [D + 1, F], f16, kind="ExternalInput").ap()
        am = nc.dram_tensor("am", [8, P, 8 * M], bf16,
                            kind="ExternalInput").ap()
        adiag = nc.dram_tensor("adiag", [1, M], i32, kind="ExternalInput").ap()
        outT = nc.dram_tensor("outT", [F, M], f32, kind="ExternalOutput").ap()
        with tile.TileContext(nc) as tc:
            nc._tc = tc
            build_kernel(nc, outT, xt, wb, am, adiag)
        nc.compile()
        _NC_CACHE["nc"] = nc
    return _NC_CACHE["nc"]


def make_in_maps(X, A, W, b):
    import ml_dtypes
    X = np.asarray(X, dtype=np.float32)
    A = np.asarray(A)
    if A.dtype != np.int32:
        A = A.astype(np.int32)
    W = np.asarray(W, dtype=np.float32)
    b = np.asarray(b, dtype=np.float32).reshape(1, F)
    wb = np.ascontiguousarray(
        np.concatenate([W, b], axis=0).astype(np.float16))      # [201, 64]
    XT = np.concatenate([X.T, np.ones((1, N), np.float32)],
                        axis=0).astype(np.float16)              # [201, N]
    rng = np.arange(M)
    in_maps = []
    for c in range(NCORES):
        r0 = c * M
        xt_c = np.ascontiguousarray(np.roll(XT, -r0, axis=1))
        blk = np.roll(A[r0 : r0 + M], -r0, axis=1).copy()       # [M, N]
        adiag = blk[rng, rng].reshape(1, M).astype(np.int32)
        blk[rng, rng] = 0            # diag handled exactly by the fp32 tail
        # [8, 128, 8192] bf16 {0,1}: group g lane u holds j = 8g+u
        bits = ((blk.T.reshape(8, 8, P, M) > 0).astype(np.uint16) * 0x3F80)
        am = np.ascontiguousarray(bits.transpose(0, 2, 1, 3)
                                  .reshape(8, P, 8 * M)).view(
                                      ml_dtypes.bfloat16)
        in_maps.append({"xt": xt_c, "wb": wb, "am": am, "adiag": adiag})
    return in_maps


def kernel(X, A, W, b):
    nc = get_compiled()
    in_maps = make_in_maps(X, A, W, b)
    res = run_bass_kernel_spmd(nc, in_maps, list(range(NCORES)))
    outTs = [res.results[c]["outT"] for c in range(NCORES)]
    return np.ascontiguousarray(np.concatenate(outTs, axis=1).T)
